# revision 30
# baseline (speedup 1.0000x reference)
"""Trainium2 Bass kernel for a capsule-network (MIND-style) interest extractor.

Math (per batch element b):
  hat[b,s,(n,d')] = sum_d u[b,s,d] * w[s, n*D+d', d]          (bilinear map)
  3 rounds of dynamic routing over s (softmax across n=4, masked), then
  interest_capsule = squash(sum_s sw*hat), readout = capsule[argmax_n <cap_n, eb>].

Distribution: pure data-parallel over batch (B=2048 -> 8 cores x 256), w replicated.

Device algorithm (fp32; per b-tile of 128 rows), key transformations:
  * The host ships y0 = 0.25*mask*u (transposed/packed) instead of u.
    Since mask is 0/1 and sw = softmax(cw)*mask, using
    hat' = 0.25*mask*hat with sw' = 4*e*recipZ (no mask) and cap4 = 4*cap
    in the delta dot-products reproduces the reference exactly while
    folding the mask multiply into host prep. u itself is never needed.
  * Routing pass fusion: softmax over n is (b,s)-local, so
    delta_i -> softmax -> E1_(i+1) accumulation all consume the SAME
    s-batch of hat' streamed through PSUM. TensorE computes hat' three
    times total (pass A: s-pair-packed matmul accumulation for cap0;
    passes B and C: per-s batches for the two routing updates).
  * Matmuls touching partition groups 0-63 vs 64-127 (s-parity packing)
    are emitted parity-grouped with parity-major PSUM layout: concurrent
    PE row-group matmuls into the same PSUM bank hard-fault on HW.
"""

import sys

import numpy as np

for _p in ("/opt/trn_rl_repo",):
    if _p not in sys.path:
        sys.path.insert(0, _p)

B, S, D, NI = 2048, 200, 64, 4
K = NI * D          # 256
NCORES = 8
BL = B // NCORES    # 256 rows per core
P = 128             # partitions / b-tile rows
NT = BL // P        # b-tiles per core (2)
J = S // 2          # s-pairs (100)
SB = 8              # s-values per PSUM batch
SBH = SB // 2


def _build_bass():
    from contextlib import ExitStack

    import concourse.bacc as bacc
    import concourse.tile as tile
    from concourse import mybir

    f32 = mybir.dt.float32
    Alu = mybir.AluOpType
    Act = mybir.ActivationFunctionType
    AxX = mybir.AxisListType.X
    AxXY = mybir.AxisListType.XY

    nc = bacc.Bacc("TRN2", target_bir_lowering=False)

    y0T2 = nc.declare_dram_parameter("y0T2", [P, J, BL], f32, isOutput=False)
    wT2 = nc.declare_dram_parameter("wT2", [P, J, K], f32, isOutput=False)
    eb_d = nc.declare_dram_parameter("eb", [BL, D], f32, isOutput=False)
    cap_d = nc.declare_dram_parameter("cap", [BL, K], f32, isOutput=True)
    ro_d = nc.declare_dram_parameter("ro", [BL, D], f32, isOutput=True)

    with tile.TileContext(nc) as tc, ExitStack() as ctx:
        singles = ctx.enter_context(tc.tile_pool(name="singles", bufs=1))
        state = ctx.enter_context(tc.tile_pool(name="state", bufs=1))
        tmps = ctx.enter_context(tc.tile_pool(name="tmps", bufs=2))
        smalls = ctx.enter_context(tc.tile_pool(name="smalls", bufs=2))
        psum = ctx.enter_context(tc.tile_pool(name="psum", bufs=2, space="PSUM"))

        w_sb = singles.tile([P, J, K], f32)
        y_sb = state.tile([P, J, P], f32)
        WCH = 5
        for j0 in range(0, J, WCH):
            nc.sync.dma_start(
                out=w_sb[:, j0 : j0 + WCH, :], in_=wT2[:, j0 : j0 + WCH, :]
            )
            nc.sync.dma_start(
                out=y_sb[:, j0 : j0 + WCH, :],
                in_=y0T2[:, j0 : j0 + WCH, 0:P],
            )
        eb_sb = state.tile([P, D], f32)
        cw = state.tile([P, NI, S], f32)
        capa = state.tile([P, NI, D], f32)
        cap = state.tile([P, NI, D], f32)
        cap4 = state.tile([P, NI, D], f32)

        def hat_batch(s0):
            """hat' for s in [s0, s0+SB) into PSUM; ps[p, q, i, k] = s0+2i+q."""
            ps = psum.tile([P, 2, SBH, K], f32, tag="ps")
            j0 = s0 // 2
            for q in (0, 1):
                for i in range(SBH):
                    nc.tensor.matmul(
                        ps[:, q, i, :],
                        y_sb[64 * q : 64 * q + 64, j0 + i, :],
                        w_sb[64 * q : 64 * q + 64, j0 + i, :],
                        start=True,
                        stop=True,
                    )
            return ps

        def squash(cin, cout, pre=1.0):
            """cout = squash(pre * cin). pre is a power of 2 (exact scaling);
            lets callers accumulate at 1/pre scale and skip a per-batch mul."""
            p2 = pre * pre
            sq = smalls.tile([P, NI, D], f32, tag="sq")
            nc.vector.tensor_mul(sq[:], cin[:], cin[:])
            r2 = smalls.tile([P, NI], f32, tag="r2")
            nc.vector.tensor_reduce(r2[:], sq[:], axis=AxX, op=Alu.add)
            t1 = smalls.tile([P, NI], f32, tag="t1")
            nc.vector.tensor_scalar(t1[:], r2[:], p2, 1.0, Alu.mult, Alu.add)
            i1 = smalls.tile([P, NI], f32, tag="i1")
            nc.vector.reciprocal(i1[:], t1[:])
            r2b = smalls.tile([P, NI], f32, tag="r2b")
            nc.vector.tensor_scalar(r2b[:], r2[:], p2, 1e-9, Alu.mult, Alu.add)
            s1 = smalls.tile([P, NI], f32, tag="s1")
            nc.scalar.activation(s1[:], r2b[:], Act.Sqrt, bias=0.0, scale=1.0)
            i2 = smalls.tile([P, NI], f32, tag="i2")
            nc.vector.reciprocal(i2[:], s1[:])
            al = smalls.tile([P, NI], f32, tag="al")
            nc.vector.tensor_mul(al[:], r2[:], i1[:])
            nc.vector.tensor_mul(al[:], al[:], i2[:])
            if pre != 1.0:
                nc.vector.tensor_scalar_mul(al[:], al[:], pre * p2)
            nc.vector.tensor_mul(
                cout[:], cin[:], al[:, :, None].broadcast_to([P, NI, D])
            )

        def routing_pass(accumulate_cw):
            """One fused pass: delta (from cap4) -> cw -> softmax -> E1 acc.

            accumulate_cw=False: cw := delta (iteration 0 -> 1)
            accumulate_cw=True:  cw += delta (iteration 1 -> 2)
            Leaves the next iteration's unsquashed capsule sum in `capa`.
            """
            nc.vector.memset(capa[:], 0.0)
            for s0 in range(0, S, SB):
                ps = hat_batch(s0)
                # stage PSUM -> SBUF on the (otherwise idle) ScalarEngine so
                # the PSUM slot frees immediately and the PE never stalls
                pss = tmps.tile([P, 2, SBH, K], f32, tag="pss", bufs=3)
                nc.scalar.copy(pss[:], ps[:])
                psv = pss[:].rearrange("p q i (n d) -> p i q n d", n=NI)
                cwv = cw[:, :, s0 : s0 + SB].rearrange("p n (i q) -> p i q n", q=2)
                # delta'' = <hat', 4*cap> = mask * delta -- reads PSUM directly,
                # concurrent with the ACT staging copy
                tmp = tmps.tile([P, SBH, 2, NI, D], f32, tag="tmp")
                nc.vector.tensor_mul(
                    tmp[:],
                    ps[:].rearrange("p q i (n d) -> p i q n d", n=NI),
                    cap4[:, None, None, :, :].broadcast_to([P, SBH, 2, NI, D]),
                )
                if accumulate_cw:
                    dlb = smalls.tile([P, SBH, 2, NI], f32, tag="dlb", bufs=4)
                    nc.vector.tensor_reduce(dlb[:], tmp[:], axis=AxX, op=Alu.add)
                    nc.vector.tensor_add(cwv, cwv, dlb[:])
                else:
                    nc.vector.tensor_reduce(cwv, tmp[:], axis=AxX, op=Alu.add)
                # softmax over n, (b,s)-local; sw' = 4*e/Z  (mask lives in hat')
                mxb = smalls.tile([P, SBH, 2], f32, tag="mxb", bufs=4)
                nc.vector.tensor_reduce(mxb[:], cwv, axis=AxX, op=Alu.max)
                ebb = smalls.tile([P, SBH, 2, NI], f32, tag="ebb", bufs=4)
                nc.vector.tensor_sub(
                    ebb[:], cwv, mxb[:, :, :, None].broadcast_to([P, SBH, 2, NI])
                )
                nc.scalar.activation(ebb[:], ebb[:], Act.Exp)
                zb = smalls.tile([P, SBH, 2], f32, tag="zb", bufs=4)
                nc.vector.tensor_reduce(zb[:], ebb[:], axis=AxX, op=Alu.add)
                rzb = smalls.tile([P, SBH, 2], f32, tag="rzb", bufs=4)
                nc.vector.reciprocal(rzb[:], zb[:])
                swb = smalls.tile([P, SBH, 2, NI], f32, tag="swb", bufs=4)
                nc.vector.tensor_mul(
                    swb[:],
                    ebb[:],
                    rzb[:, :, :, None].broadcast_to([P, SBH, 2, NI]),
                )
                # E1 accumulation for the next iteration's capsule
                tmp2 = tmps.tile([P, SBH, 2, NI, D], f32, tag="tmp")
                nc.vector.tensor_mul(
                    tmp2[:],
                    psv,
                    swb[:, :, :, :, None].broadcast_to([P, SBH, 2, NI, D]),
                )
                red = smalls.tile([P, NI, D], f32, tag="red", bufs=4)
                nc.vector.tensor_reduce(
                    red[:], tmp2[:].transpose([0, 3, 4, 1, 2]), axis=AxXY,
                    op=Alu.add,
                )
                nc.vector.tensor_add(capa[:], capa[:], red[:])

        for t in range(NT):
            YCH = 5
            if t > 0:
                for j0 in range(0, J, YCH):
                    nc.sync.dma_start(
                        out=y_sb[:, j0 : j0 + YCH, :],
                        in_=y0T2[:, j0 : j0 + YCH, t * P : t * P + P],
                    )
            nc.sync.dma_start(out=eb_sb[:], in_=eb_d[t * P : t * P + P, :])

            # ---- pass A: cap0 = squash(sum_s hat'_s), pure PE accumulation
            ps_a = psum.tile([P, 2, SBH, K], f32, tag="ps")
            pa = ps_a[:, 0, 0, :]
            for j in range(J):
                nc.tensor.matmul(
                    pa,
                    y_sb[:, j, :],
                    w_sb[:, j, :],
                    start=(j == 0),
                    stop=(j == J - 1),
                )
            nc.vector.tensor_copy(capa[:].rearrange("p n d -> p (n d)"), pa)
            squash(capa, cap)
            nc.vector.tensor_scalar_mul(cap4[:], cap[:], 4.0)

            # ---- pass B: delta0 -> cw -> sw1 -> E1(iter1); pass C likewise
            routing_pass(accumulate_cw=False)
            squash(capa, cap, pre=4.0)
            nc.vector.tensor_scalar_mul(cap4[:], cap[:], 4.0)
            routing_pass(accumulate_cw=True)
            squash(capa, cap, pre=4.0)

            # ---- hard readout (argmax over 4 logits; softmax is monotonic)
            pr = smalls.tile([P, NI, D], f32, tag="pr")
            nc.vector.tensor_mul(
                pr[:], cap[:], eb_sb[:, None, :].broadcast_to([P, NI, D])
            )
            dt = smalls.tile([P, NI], f32, tag="dt")
            nc.vector.tensor_reduce(dt[:], pr[:], axis=AxX, op=Alu.add)
            mx1 = smalls.tile([P, 1], f32, tag="mx1")
            nc.vector.tensor_reduce(mx1[:], dt[:], axis=AxX, op=Alu.max)
            g = smalls.tile([P, NI], f32, tag="g")
            nc.vector.tensor_tensor(
                g[:], dt[:], mx1[:].broadcast_to([P, NI]), op=Alu.is_ge
            )
            notk = smalls.tile([P, 1], f32, tag="notk")
            sel = smalls.tile([P, NI], f32, tag="sel")
            nc.vector.tensor_copy(sel[:, 0:1], g[:, 0:1])
            nc.vector.tensor_scalar(
                notk[:], g[:, 0:1], -1.0, 1.0, Alu.mult, Alu.add
            )
            for n in range(1, NI):
                nc.vector.tensor_mul(sel[:, n : n + 1], g[:, n : n + 1], notk[:])
                if n < NI - 1:
                    t2 = smalls.tile([P, 1], f32, tag="t2")
                    nc.vector.tensor_scalar(
                        t2[:], sel[:, n : n + 1], -1.0, 1.0, Alu.mult, Alu.add
                    )
                    nc.vector.tensor_mul(notk[:], notk[:], t2[:])
            ro = smalls.tile([P, D], f32, tag="ro")
            nc.vector.tensor_scalar_mul(ro[:], cap[:, 0, :], sel[:, 0:1])
            for n in range(1, NI):
                nc.vector.scalar_tensor_tensor(
                    out=ro[:],
                    in0=cap[:, n, :],
                    scalar=sel[:, n : n + 1],
                    in1=ro[:],
                    op0=Alu.mult,
                    op1=Alu.add,
                )

            nc.sync.dma_start(
                out=cap_d[t * P : t * P + P, :],
                in_=cap[:].rearrange("p n d -> p (n d)"),
            )
            nc.sync.dma_start(out=ro_d[t * P : t * P + P, :], in_=ro[:])

    nc.finalize()
    return nc


_NC_CACHE = None


def _get_nc():
    global _NC_CACHE
    if _NC_CACHE is None:
        _NC_CACHE = _build_bass()
    return _NC_CACHE


def _pack_inputs(item_his_emb, item_eb, mask, w):
    u = np.asarray(item_his_emb, dtype=np.float32)
    eb = np.ascontiguousarray(np.asarray(item_eb, dtype=np.float32))
    mk = np.asarray(mask, dtype=np.float32)
    ww = np.asarray(w, dtype=np.float32)[0]  # [S, K, D]
    # wT2[(q,d), j, k] with s = 2j+q
    wT2 = np.ascontiguousarray(
        ww.reshape(J, 2, K, D).transpose(1, 3, 0, 2).reshape(P, J, K)
    )
    y0 = (0.25 * mk)[:, :, None] * u  # [B, S, D]
    in_maps = []
    for c in range(NCORES):
        yl = y0[c * BL : (c + 1) * BL]  # [BL, S, D]
        y0T2 = np.ascontiguousarray(
            yl.reshape(BL, J, 2, D).transpose(2, 3, 1, 0).reshape(P, J, BL)
        )
        in_maps.append(
            {
                "y0T2": y0T2,
                "wT2": wT2,
                "eb": np.ascontiguousarray(eb[c * BL : (c + 1) * BL]),
            }
        )
    return in_maps


def run(inputs, trace=False, **spmd_kwargs):
    from concourse.bass_utils import run_bass_kernel_spmd

    in_maps = _pack_inputs(**inputs)
    nc = _get_nc()
    res = run_bass_kernel_spmd(
        nc, in_maps, core_ids=list(range(NCORES)), trace=trace, **spmd_kwargs
    )
    caps = np.concatenate([res.results[c]["cap"] for c in range(NCORES)], axis=0)
    ros = np.concatenate([res.results[c]["ro"] for c in range(NCORES)], axis=0)
    return (caps.reshape(B, NI, D), ros), res


def kernel(**inputs):
    (caps, ros), _ = run(inputs, trace=False)
    return caps, ros


if __name__ == "__main__":
    rng = np.random.default_rng(0)
    ins = {
        "item_his_emb": rng.standard_normal((B, S, D), dtype=np.float32),
        "item_eb": rng.standard_normal((B, D), dtype=np.float32),
        "mask": rng.integers(0, 2, (B, S)).astype(np.float32),
        "w": rng.standard_normal((1, S, K, D), dtype=np.float32),
    }
    out, _ = run(ins)
    print([o.shape for o in out])


# revision 31
# speedup vs baseline: 1.0012x; 1.0012x over previous
"""Trainium2 Bass kernel for a capsule-network (MIND-style) interest extractor.

Math (per batch element b):
  hat[b,s,(n,d')] = sum_d u[b,s,d] * w[s, n*D+d', d]          (bilinear map)
  3 rounds of dynamic routing over s (softmax across n=4, masked), then
  interest_capsule = squash(sum_s sw*hat), readout = capsule[argmax_n <cap_n, eb>].

Distribution: pure data-parallel over batch (B=2048 -> 8 cores x 256), w replicated.

Device algorithm (fp32; per b-tile of 128 rows), key transformations:
  * The host ships y0 = 0.25*mask*u (transposed/packed) instead of u.
    Since mask is 0/1 and sw = softmax(cw)*mask, using
    hat' = 0.25*mask*hat with sw' = 4*e*recipZ (no mask) and cap4 = 4*cap
    in the delta dot-products reproduces the reference exactly while
    folding the mask multiply into host prep. u itself is never needed.
  * Routing pass fusion: softmax over n is (b,s)-local, so
    delta_i -> softmax -> E1_(i+1) accumulation all consume the SAME
    s-batch of hat' streamed through PSUM. TensorE computes hat' three
    times total (pass A: s-pair-packed matmul accumulation for cap0;
    passes B and C: per-s batches for the two routing updates).
  * Matmuls touching partition groups 0-63 vs 64-127 (s-parity packing)
    are emitted parity-grouped with parity-major PSUM layout: concurrent
    PE row-group matmuls into the same PSUM bank hard-fault on HW.
"""

import sys

import numpy as np

for _p in ("/opt/trn_rl_repo",):
    if _p not in sys.path:
        sys.path.insert(0, _p)

B, S, D, NI = 2048, 200, 64, 4
K = NI * D          # 256
NCORES = 8
BL = B // NCORES    # 256 rows per core
P = 128             # partitions / b-tile rows
NT = BL // P        # b-tiles per core (2)
J = S // 2          # s-pairs (100)
SB = 8              # s-values per PSUM batch
SBH = SB // 2


def _build_bass():
    from contextlib import ExitStack

    import concourse.bacc as bacc
    import concourse.tile as tile
    from concourse import mybir

    f32 = mybir.dt.float32
    Alu = mybir.AluOpType
    Act = mybir.ActivationFunctionType
    AxX = mybir.AxisListType.X
    AxXY = mybir.AxisListType.XY

    nc = bacc.Bacc("TRN2", target_bir_lowering=False)

    y0T2 = nc.declare_dram_parameter("y0T2", [P, J, BL], f32, isOutput=False)
    wT2 = nc.declare_dram_parameter("wT2", [P, J, K], f32, isOutput=False)
    eb_d = nc.declare_dram_parameter("eb", [BL, D], f32, isOutput=False)
    cap_d = nc.declare_dram_parameter("cap", [BL, K], f32, isOutput=True)
    ro_d = nc.declare_dram_parameter("ro", [BL, D], f32, isOutput=True)

    with tile.TileContext(nc) as tc, ExitStack() as ctx:
        singles = ctx.enter_context(tc.tile_pool(name="singles", bufs=1))
        state = ctx.enter_context(tc.tile_pool(name="state", bufs=1))
        tmps = ctx.enter_context(tc.tile_pool(name="tmps", bufs=2))
        smalls = ctx.enter_context(tc.tile_pool(name="smalls", bufs=2))
        psum = ctx.enter_context(tc.tile_pool(name="psum", bufs=2, space="PSUM"))

        w_sb = singles.tile([P, J, K], f32)
        y_sb = state.tile([P, J, P], f32)
        WCH = 5
        for j0 in range(0, J, WCH):
            nc.sync.dma_start(
                out=w_sb[:, j0 : j0 + WCH, :], in_=wT2[:, j0 : j0 + WCH, :]
            )
            nc.sync.dma_start(
                out=y_sb[:, j0 : j0 + WCH, :],
                in_=y0T2[:, j0 : j0 + WCH, 0:P],
            )
        eb_sb = state.tile([P, D], f32)
        cw = state.tile([P, NI, S], f32)
        capa = state.tile([P, NI, D], f32)
        cap = state.tile([P, NI, D], f32)
        cap4 = state.tile([P, NI, D], f32)

        def hat_batch(s0):
            """hat' for s in [s0, s0+SB) into PSUM; ps[p, q, i, k] = s0+2i+q."""
            ps = psum.tile([P, 2, SBH, K], f32, tag="ps")
            j0 = s0 // 2
            for q in (0, 1):
                for i in range(SBH):
                    nc.tensor.matmul(
                        ps[:, q, i, :],
                        y_sb[64 * q : 64 * q + 64, j0 + i, :],
                        w_sb[64 * q : 64 * q + 64, j0 + i, :],
                        start=True,
                        stop=True,
                    )
            return ps

        def squash(cin, cout, pre=1.0):
            """cout = squash(pre * cin). pre is a power of 2 (exact scaling);
            lets callers accumulate at 1/pre scale and skip a per-batch mul."""
            p2 = pre * pre
            sq = smalls.tile([P, NI, D], f32, tag="sq")
            nc.vector.tensor_mul(sq[:], cin[:], cin[:])
            r2 = smalls.tile([P, NI], f32, tag="r2")
            nc.vector.tensor_reduce(r2[:], sq[:], axis=AxX, op=Alu.add)
            t1 = smalls.tile([P, NI], f32, tag="t1")
            nc.vector.tensor_scalar(t1[:], r2[:], p2, 1.0, Alu.mult, Alu.add)
            i1 = smalls.tile([P, NI], f32, tag="i1")
            nc.vector.reciprocal(i1[:], t1[:])
            r2b = smalls.tile([P, NI], f32, tag="r2b")
            nc.vector.tensor_scalar(r2b[:], r2[:], p2, 1e-9, Alu.mult, Alu.add)
            s1 = smalls.tile([P, NI], f32, tag="s1")
            nc.scalar.activation(s1[:], r2b[:], Act.Sqrt, bias=0.0, scale=1.0)
            i2 = smalls.tile([P, NI], f32, tag="i2")
            nc.vector.reciprocal(i2[:], s1[:])
            al = smalls.tile([P, NI], f32, tag="al")
            nc.vector.tensor_mul(al[:], r2[:], i1[:])
            nc.vector.tensor_mul(al[:], al[:], i2[:])
            if pre != 1.0:
                nc.vector.tensor_scalar_mul(al[:], al[:], pre * p2)
            nc.vector.tensor_mul(
                cout[:], cin[:], al[:, :, None].broadcast_to([P, NI, D])
            )

        def routing_pass(accumulate_cw):
            """One fused pass: delta (from cap4) -> cw -> softmax -> E1 acc.

            accumulate_cw=False: cw := delta (iteration 0 -> 1)
            accumulate_cw=True:  cw += delta (iteration 1 -> 2)
            Leaves the next iteration's unsquashed capsule sum in `capa`.
            """
            for s0 in range(0, S, SB):
                ps = hat_batch(s0)
                # stage PSUM -> SBUF on the (otherwise idle) ScalarEngine so
                # the PSUM slot frees immediately and the PE never stalls
                pss = tmps.tile([P, 2, SBH, K], f32, tag="pss", bufs=3)
                nc.scalar.copy(pss[:], ps[:])
                psv = pss[:].rearrange("p q i (n d) -> p i q n d", n=NI)
                cwv = cw[:, :, s0 : s0 + SB].rearrange("p n (i q) -> p i q n", q=2)
                # delta'' = <hat', 4*cap> = mask * delta -- reads PSUM directly,
                # concurrent with the ACT staging copy
                tmp = tmps.tile([P, SBH, 2, NI, D], f32, tag="tmp")
                nc.vector.tensor_mul(
                    tmp[:],
                    ps[:].rearrange("p q i (n d) -> p i q n d", n=NI),
                    cap4[:, None, None, :, :].broadcast_to([P, SBH, 2, NI, D]),
                )
                if accumulate_cw:
                    dlb = smalls.tile([P, SBH, 2, NI], f32, tag="dlb", bufs=4)
                    nc.vector.tensor_reduce(dlb[:], tmp[:], axis=AxX, op=Alu.add)
                    nc.vector.tensor_add(cwv, cwv, dlb[:])
                else:
                    nc.vector.tensor_reduce(cwv, tmp[:], axis=AxX, op=Alu.add)
                # softmax over n, (b,s)-local; sw' = 4*e/Z  (mask lives in hat')
                mxb = smalls.tile([P, SBH, 2], f32, tag="mxb", bufs=4)
                nc.vector.tensor_reduce(mxb[:], cwv, axis=AxX, op=Alu.max)
                ebb = smalls.tile([P, SBH, 2, NI], f32, tag="ebb", bufs=4)
                nc.vector.tensor_sub(
                    ebb[:], cwv, mxb[:, :, :, None].broadcast_to([P, SBH, 2, NI])
                )
                nc.scalar.activation(ebb[:], ebb[:], Act.Exp)
                zb = smalls.tile([P, SBH, 2], f32, tag="zb", bufs=4)
                nc.vector.tensor_reduce(zb[:], ebb[:], axis=AxX, op=Alu.add)
                rzb = smalls.tile([P, SBH, 2], f32, tag="rzb", bufs=4)
                nc.vector.reciprocal(rzb[:], zb[:])
                swb = smalls.tile([P, SBH, 2, NI], f32, tag="swb", bufs=4)
                nc.vector.tensor_mul(
                    swb[:],
                    ebb[:],
                    rzb[:, :, :, None].broadcast_to([P, SBH, 2, NI]),
                )
                # E1 accumulation for the next iteration's capsule
                tmp2 = tmps.tile([P, SBH, 2, NI, D], f32, tag="tmp")
                nc.vector.tensor_mul(
                    tmp2[:],
                    psv,
                    swb[:, :, :, :, None].broadcast_to([P, SBH, 2, NI, D]),
                )
                if s0 == 0:
                    # first batch writes the accumulator directly
                    nc.vector.tensor_reduce(
                        capa[:], tmp2[:].transpose([0, 3, 4, 1, 2]), axis=AxXY,
                        op=Alu.add,
                    )
                else:
                    red = smalls.tile([P, NI, D], f32, tag="red", bufs=4)
                    nc.vector.tensor_reduce(
                        red[:], tmp2[:].transpose([0, 3, 4, 1, 2]), axis=AxXY,
                        op=Alu.add,
                    )
                    nc.vector.tensor_add(capa[:], capa[:], red[:])

        for t in range(NT):
            YCH = 5
            if t > 0:
                for j0 in range(0, J, YCH):
                    nc.sync.dma_start(
                        out=y_sb[:, j0 : j0 + YCH, :],
                        in_=y0T2[:, j0 : j0 + YCH, t * P : t * P + P],
                    )
            nc.sync.dma_start(out=eb_sb[:], in_=eb_d[t * P : t * P + P, :])

            # ---- pass A: cap0 = squash(sum_s hat'_s), pure PE accumulation
            ps_a = psum.tile([P, 2, SBH, K], f32, tag="ps")
            pa = ps_a[:, 0, 0, :]
            for j in range(J):
                nc.tensor.matmul(
                    pa,
                    y_sb[:, j, :],
                    w_sb[:, j, :],
                    start=(j == 0),
                    stop=(j == J - 1),
                )
            nc.vector.tensor_copy(capa[:].rearrange("p n d -> p (n d)"), pa)
            squash(capa, cap)
            nc.vector.tensor_scalar_mul(cap4[:], cap[:], 4.0)

            # ---- pass B: delta0 -> cw -> sw1 -> E1(iter1); pass C likewise
            routing_pass(accumulate_cw=False)
            squash(capa, cap, pre=4.0)
            nc.vector.tensor_scalar_mul(cap4[:], cap[:], 4.0)
            routing_pass(accumulate_cw=True)
            squash(capa, cap, pre=4.0)

            # ---- hard readout (argmax over 4 logits; softmax is monotonic)
            pr = smalls.tile([P, NI, D], f32, tag="pr")
            nc.vector.tensor_mul(
                pr[:], cap[:], eb_sb[:, None, :].broadcast_to([P, NI, D])
            )
            dt = smalls.tile([P, NI], f32, tag="dt")
            nc.vector.tensor_reduce(dt[:], pr[:], axis=AxX, op=Alu.add)
            mx1 = smalls.tile([P, 1], f32, tag="mx1")
            nc.vector.tensor_reduce(mx1[:], dt[:], axis=AxX, op=Alu.max)
            g = smalls.tile([P, NI], f32, tag="g")
            nc.vector.tensor_tensor(
                g[:], dt[:], mx1[:].broadcast_to([P, NI]), op=Alu.is_ge
            )
            notk = smalls.tile([P, 1], f32, tag="notk")
            sel = smalls.tile([P, NI], f32, tag="sel")
            nc.vector.tensor_copy(sel[:, 0:1], g[:, 0:1])
            nc.vector.tensor_scalar(
                notk[:], g[:, 0:1], -1.0, 1.0, Alu.mult, Alu.add
            )
            for n in range(1, NI):
                nc.vector.tensor_mul(sel[:, n : n + 1], g[:, n : n + 1], notk[:])
                if n < NI - 1:
                    t2 = smalls.tile([P, 1], f32, tag="t2")
                    nc.vector.tensor_scalar(
                        t2[:], sel[:, n : n + 1], -1.0, 1.0, Alu.mult, Alu.add
                    )
                    nc.vector.tensor_mul(notk[:], notk[:], t2[:])
            ro = smalls.tile([P, D], f32, tag="ro")
            nc.vector.tensor_scalar_mul(ro[:], cap[:, 0, :], sel[:, 0:1])
            for n in range(1, NI):
                nc.vector.scalar_tensor_tensor(
                    out=ro[:],
                    in0=cap[:, n, :],
                    scalar=sel[:, n : n + 1],
                    in1=ro[:],
                    op0=Alu.mult,
                    op1=Alu.add,
                )

            nc.sync.dma_start(
                out=cap_d[t * P : t * P + P, :],
                in_=cap[:].rearrange("p n d -> p (n d)"),
            )
            nc.sync.dma_start(out=ro_d[t * P : t * P + P, :], in_=ro[:])

    nc.finalize()
    return nc


_NC_CACHE = None


def _get_nc():
    global _NC_CACHE
    if _NC_CACHE is None:
        _NC_CACHE = _build_bass()
    return _NC_CACHE


def _pack_inputs(item_his_emb, item_eb, mask, w):
    u = np.asarray(item_his_emb, dtype=np.float32)
    eb = np.ascontiguousarray(np.asarray(item_eb, dtype=np.float32))
    mk = np.asarray(mask, dtype=np.float32)
    ww = np.asarray(w, dtype=np.float32)[0]  # [S, K, D]
    # wT2[(q,d), j, k] with s = 2j+q
    wT2 = np.ascontiguousarray(
        ww.reshape(J, 2, K, D).transpose(1, 3, 0, 2).reshape(P, J, K)
    )
    y0 = (0.25 * mk)[:, :, None] * u  # [B, S, D]
    in_maps = []
    for c in range(NCORES):
        yl = y0[c * BL : (c + 1) * BL]  # [BL, S, D]
        y0T2 = np.ascontiguousarray(
            yl.reshape(BL, J, 2, D).transpose(2, 3, 1, 0).reshape(P, J, BL)
        )
        in_maps.append(
            {
                "y0T2": y0T2,
                "wT2": wT2,
                "eb": np.ascontiguousarray(eb[c * BL : (c + 1) * BL]),
            }
        )
    return in_maps


def run(inputs, trace=False, **spmd_kwargs):
    from concourse.bass_utils import run_bass_kernel_spmd

    in_maps = _pack_inputs(**inputs)
    nc = _get_nc()
    res = run_bass_kernel_spmd(
        nc, in_maps, core_ids=list(range(NCORES)), trace=trace, **spmd_kwargs
    )
    caps = np.concatenate([res.results[c]["cap"] for c in range(NCORES)], axis=0)
    ros = np.concatenate([res.results[c]["ro"] for c in range(NCORES)], axis=0)
    return (caps.reshape(B, NI, D), ros), res


def kernel(**inputs):
    (caps, ros), _ = run(inputs, trace=False)
    return caps, ros


if __name__ == "__main__":
    rng = np.random.default_rng(0)
    ins = {
        "item_his_emb": rng.standard_normal((B, S, D), dtype=np.float32),
        "item_eb": rng.standard_normal((B, D), dtype=np.float32),
        "mask": rng.integers(0, 2, (B, S)).astype(np.float32),
        "w": rng.standard_normal((1, S, K, D), dtype=np.float32),
    }
    out, _ = run(ins)
    print([o.shape for o in out])
